# revision 3
# baseline (speedup 1.0000x reference)
"""Time-varying 33-tap FIR low-pass filter on 8 Trainium2 NeuronCores.

y[b,t] = sum_u filt[t,u] * x[b, t+u-16],  filt = host-computed windowed-sinc
bank (n,33) derived from scalars alpha/beta (tiny; O(n*33) host FLOPs).

Sharding: time dim split across the 8 cores (4096 t-columns each, all 64
batch rows).  Per core the banded matmul y = x @ W (contraction over input
time s) is tiled into 22 TensorE matmuls.  Each matmul packs TWO 128-sample
x-chunks, offset by 96 samples, side by side in the stationary operand
(K=128, M=128 = 2 halves x 64 batch).  The 96-offset makes every output
column's 33-tap band land entirely inside one half, so each PSUM column is
valid in exactly one 64-row half and the chunk serves 192 output columns
with no accumulation pass:

  lhsT[k, 64*h + b] = x[b, S + 96*h + k]           (S = core_t0 - 16 + 192*j)
  rhs [k, n]        = filt[S+16+n, u] at k = (n % 96) + u   (zeros elsewhere)
  psum[64*h(n) + b, n] = y[b, S+16+n],   h(n) = n // 96

Measured structure (profile: exec window = first body instruction ->
last instruction, which INCLUDES the NRT postamble -- a fixed ~8.4us of
walrus exit barrier + 253 per-semaphore clears split across the 5 engines
(PE slowest, ~115ns each) + final barriers.  The controllable span is:
~1.5us body-start latency, the input DMA stream (~1.7MB at ~270GB/s --
a structural floor: the banded moving operand costs K*2B = 256B per
output column regardless of band content, and every on-device band-
generation scheme dies on either the partition-broadcast problem, per-op
engine fixed overheads, or DMA packet granularity), then a dependency
tail: last-input-byte -> completion sem (~0.5-0.9us) -> last matmuls ->
extraction -> output-DMA generation (~0.6us) -> DGE start delay (~0.66us)
-> transfer.

Schedule optimizations over the 22.2us/20.8us baseline:
- all matmul operands bf16; input partition-major, few LARGE grouped DMAs
  (as before)
- activation-table preload: a dummy scalar Copy at body start hoists the
  1.28us ACT_TABLE_LOAD off the first extraction's critical path
- PROCESSING ORDER [0..11, 21, 12..20]: the partial chunk (21: only 64
  valid outputs, 64-col matmul) rides mid-stream inside input group G1,
  so the final chain after the last input byte is exactly one full chunk
  (20): one matmul -> one pair of extractions -> one small gen+transfer
- finer input groups at the tail ((4),(4-11,21),(12-17),(18-19),(20)) so
  completion semaphores fire progressively and the PE never bunches
- output-DMA generations spread across THREE issue engines (gpsimd SWDGE
  for the first group, Sync and Scalar HWDGE alternating for the rest) --
  the baseline serialized all gens on Sync (~600ns each + 660ns DGE
  delay), which alone stretched the tail by ~2us
- st staging keeps half1 in partitions 64:128 (128-line output DMAs);
  host unfolds positions->chunks and upcasts
Fixed costs that bound further gains: the NRT postamble (~7.3us: 253 sem
clears + barriers -- def.json already declares runtime_semaphore_count=3,
the reset length is runtime hygiene, not kernel-controllable), ~0.7us
walrus entry/register preamble, ~0.9us DMA completion-semaphore latency,
~1.25us gen+DGE-delay on the final output group.
"""

import sys
from contextlib import ExitStack

import numpy as np
import ml_dtypes

if "/opt/trn_rl_repo" not in sys.path:
    sys.path.insert(0, "/opt/trn_rl_repo")

from concourse import bass, mybir
from concourse.bass_utils import run_bass_kernel_spmd

N = 32768          # time length
B = 64             # batch
NCORES = 8
TCORE = N // NCORES            # 4096 output columns per core
CT = 192                       # output columns served per chunk
NJ = (TCORE + CT - 1) // CT    # 22 chunks per core (last one partial: 64 cols)
KP = 128                       # contraction rows per matmul
TAPS = 33
HALF = 16
W = 128 + CT                   # 320 columns per [stationary | moving] chunk

# processing order: partial chunk 21 (64 valid cols) runs mid-stream so the
# final dependency chain covers exactly one chunk (20)
P_ORDER = list(range(0, 12)) + [21] + list(range(12, 21))
P_INV = [0] * NJ
for _p, _j in enumerate(P_ORDER):
    P_INV[_j] = _p

# input groups: lists of xwt column ranges (in chunk units), one semaphore
# each.  G1 carries chunk 21's stationary+64-col band (192 cols -- its last
# 128 band cols are all-zero padding and are never read).  Tail groups are
# small so their completion sems fire progressively.
IN_GROUPS = (
    ((0, 4, W),),                              # G0: chunks 0-3
    ((4, 12, W), (21, 22, 192)),               # G1: chunks 4-11 + chunk 21
    ((12, 18, W),),                            # G2: chunks 12-17
    ((18, 20, W),),                            # G3: chunks 18-19
    ((20, 21, W),),                            # G4: chunk 20 (stream tail)
)
# input group index gating each POSITION's matmul
POS_GROUP = [0] * 4 + [1] * 9 + [2] * 6 + [3] * 2 + [4]
# extraction units over positions: quads while the stream runs, pairs and
# singles at the tail so the last unit (position 21 = chunk 20) is minimal
EX_UNITS = ((0, 4), (4, 8), (8, 12), (12, 14), (14, 16), (16, 18), (18, 20),
            (20, 21), (21, 22))
# output groups over positions: (pos0, pos1, unit threshold, issue engine)
# gens spread across gpsimd SWDGE / Sync HWDGE / Scalar(Act) HWDGE so the
# tail generations never serialize on one sequencer
OUT_GROUPS = (
    (0, 4, 1, "gpsimd"),
    (4, 12, 3, "sync"),
    (12, 16, 5, "scalar"),   # includes position 12 = chunk 21
    (16, 20, 7, "sync"),
    (20, 21, 8, "sync"),
    (21, 22, 9, "scalar"),   # final: chunk 20 alone, 24KB
)

_prog_cache = None


def _filters_np(alpha, beta):
    """Numpy port of reference._filters (returns the flipped bank)."""
    t = np.arange(N, dtype=np.float64)
    cutoff = (np.pi / 4.0 + float(alpha) * np.sin(float(beta) * t / 8000.0)) / (
        2.0 * np.pi
    )
    k = np.arange(TAPS, dtype=np.float64)
    window = 0.5 - 0.5 * np.cos(2.0 * np.pi * k / (TAPS - 1.0))
    tvec = np.arange(-HALF, HALF + 1, dtype=np.float64)
    arg = 2.0 * np.pi * cutoff[:, None] * tvec[None, :]
    safe = np.where(arg == 0.0, 1.0, arg)
    sinc = np.where(arg == 0.0, 1.0, np.sin(safe) / safe)
    f = 2.0 * cutoff[:, None] * window[None, :] * sinc
    f = f / f.sum(axis=-1, keepdims=True)
    return np.ascontiguousarray(f[:, ::-1]).astype(np.float32)


def _prep_inputs(x, alpha, beta):
    """Build per-core [KP, NJ*W] bf16 [stationary | banded-filter] tiles."""
    filt = _filters_np(alpha, beta)  # (N, 33)

    pad = 16 + N + 512
    xp = np.zeros((B, pad), dtype=np.float32)
    xp[:, 16 : 16 + N] = x
    xp = xp.astype(ml_dtypes.bfloat16)
    fp = np.zeros((N + 512, TAPS), dtype=ml_dtypes.bfloat16)
    fp[:N] = filt.astype(ml_dtypes.bfloat16)

    c = np.arange(NCORES)[:, None, None, None]
    j = np.arange(NJ)[None, :, None, None]
    h = np.arange(2)[None, None, :, None]
    k = np.arange(KP)[None, None, None, :]
    # global s = TCORE*c - 16 + CT*j + 96*h + k ; +16 shifts into xp coords
    sidx = TCORE * c + CT * j + 96 * h + k
    xw = xp[:, sidx]  # (B, NCORES, NJ, 2, KP)
    xw = np.ascontiguousarray(
        np.transpose(xw, (1, 2, 4, 3, 0)).reshape(NCORES, NJ, KP, 128)
    )

    u = np.arange(TAPS)[:, None]  # (33, 1)
    nn = np.arange(CT)[None, :]  # (1, 192)
    rows = (nn % 96) + u  # (33, 192) target partition rows
    cols = np.broadcast_to(nn, (TAPS, CT))
    tg = (
        TCORE * np.arange(NCORES)[:, None, None]
        + CT * np.arange(NJ)[None, :, None]
        + np.arange(CT)[None, None, :]
    )  # (NCORES, NJ, 192) global output t per column
    vals = np.transpose(fp[tg], (0, 1, 3, 2))  # (NCORES, NJ, 33, 192)
    wt = np.zeros((NCORES, NJ, KP, CT), dtype=ml_dtypes.bfloat16)
    wt[:, :, rows, cols] = vals

    # one combined [stationary | moving] tile per chunk, then partition-major
    # ([KP, NJ, W]) so grouped input DMAs move long contiguous lines
    xwt = np.concatenate([xw, wt], axis=3)  # (NCORES, NJ, KP, W)
    xwt = np.transpose(xwt, (0, 2, 1, 3)).reshape(NCORES, KP, NJ * W)
    return np.ascontiguousarray(xwt)


def _build_program():
    """Raw Bass (no Tile): walrus permits a single sync-wait slot per Matmult
    and per DMA descriptor, so waits are emitted as standalone EventSemaphore
    instructions on each engine's queue instead."""
    # Skip the const-AP publish barrier at the tail of Bass.__init__: this
    # kernel never reads const_aps (scalar Copy keeps a float bias), the NRT
    # pseudo-barrier earlier in init already rendezvoused the engines, and
    # per-sem waits order everything else.  Saves ~0.3us of preamble.
    orig_aeb = bass.Bass.all_engine_barrier
    bass.Bass.all_engine_barrier = lambda self, *, sem_only=False: None
    try:
        nc = bass.Bass(trn_type="TRN2", debug=False)
    finally:
        bass.Bass.all_engine_barrier = orig_aeb
    f32 = mybir.dt.float32
    bf16 = mybir.dt.bfloat16
    xwt_d = nc.dram_tensor("xwt", [KP, NJ * W], bf16, kind="ExternalInput").ap()
    y_d = nc.dram_tensor("y", [2 * B, NJ * 96], bf16, kind="ExternalOutput").ap()

    def unit_of(p):
        for u, (c0, c1) in enumerate(EX_UNITS):
            if c0 <= p < c1:
                return u

    with ExitStack() as ctx:
        xts = ctx.enter_context(nc.sbuf_tensor("xts", [KP, NJ * W], bf16))
        # staging keeps half1 in partitions 64:128 (no partition fold), so
        # output DMAs move 128 lines instead of 64 -- better engine pipelining
        st = ctx.enter_context(nc.sbuf_tensor("st", [2 * B, NJ, 96], bf16))
        # tiny scratch for the activation-table preload copy
        scr = ctx.enter_context(nc.sbuf_tensor("scr", [1, 2], bf16))
        # 2 quad-tensors of 4 PSUM banks each (slot = 512 fp32 = one bank);
        # extraction reads all slots of a quad in one 3D-AP op
        pps = [
            ctx.enter_context(nc.psum_tensor(f"pp{i}", [128, 4, 512], f32))
            for i in range(2)
        ]
        # one semaphore per input DMA group (no reuse -> no guards needed)
        sem_in = [
            ctx.enter_context(nc.semaphore(f"s_in{i}"))
            for i in range(len(IN_GROUPS))
        ]
        sem_pe = ctx.enter_context(nc.semaphore("s_pe"))
        sem_dve = ctx.enter_context(nc.semaphore("s_dve"))
        sem_act = ctx.enter_context(nc.semaphore("s_act"))
        sem_out = [
            ctx.enter_context(nc.semaphore(f"s_out{i}"))
            for i in range(len(OUT_GROUPS))
        ]
        block_cm = nc.Block()
        block = block_cm.__enter__()

        # expected completion value per input group (16 per dma_start)
        in_target = [16 * len(g) for g in IN_GROUPS]

        def out_dma(eng, gi):
            p0, p1, uth, _ = OUT_GROUPS[gi]
            eng.wait_ge(sem_dve, uth)
            eng.wait_ge(sem_act, uth)
            eng.dma_start(
                out=y_d[:, 96 * p0 : 96 * p1], in_=st[:, p0:p1, :]
            ).then_inc(sem_out[gi], 16)

        @block.sync
        def _(sync):
            # all input groups from one engine: generation order == transfer
            # order, so group 0 always reaches the DMA engines first
            for g, ranges in enumerate(IN_GROUPS):
                for (j0, j1, w1) in ranges:
                    sync.dma_start(
                        out=xts[:, W * j0 : W * (j1 - 1) + w1],
                        in_=xwt_d[:, W * j0 : W * (j1 - 1) + w1],
                    ).then_inc(sem_in[g], 16)
            for gi, og in enumerate(OUT_GROUPS):
                if og[3] == "sync":
                    out_dma(sync, gi)

        @block.tensor
        def _(tensor):
            reuse_th = 0
            seen_group = -1
            for p, j in enumerate(P_ORDER):
                g = POS_GROUP[p]
                if g != seen_group:
                    tensor.wait_ge(sem_in[g], in_target[g])
                    seen_group = g
                if p >= 8:
                    # PSUM slot of position p (bank p%8) is free once the unit
                    # holding position p-8 has BOTH half-copies retired
                    th = unit_of(p - 8) + 1
                    if th > reuse_th:
                        tensor.wait_ge(sem_dve, th)
                        tensor.wait_ge(sem_act, th)
                        reuse_th = th
                ncols = 64 if j == NJ - 1 else CT
                tensor.matmul(
                    pps[(p // 4) % 2].ap()[:, p % 4, 0:ncols],
                    xts[:, W * j : W * j + 128],
                    xts[:, W * j + 128 : W * j + 128 + ncols],
                    start=True,
                    stop=True,
                ).then_inc(sem_pe, 1)

        @block.vector
        def _(vector):
            # half0: outputs 0:96 of each position live in PSUM partitions
            # 0:64; one 3D-AP op extracts a whole unit.  For position 12
            # (chunk 21, 64-col matmul) columns 64:96 read stale PSUM -- the
            # host drops those outputs (beyond TCORE).
            for u, (c0, c1) in enumerate(EX_UNITS):
                vector.wait_ge(sem_pe, c1)
                vector.tensor_copy(
                    st[0:B, c0:c1, 0:96],
                    pps[(c0 // 4) % 2].ap()[0:B, c0 % 4 : c0 % 4 + c1 - c0, 0:96],
                ).then_inc(sem_dve, 1)

        @block.scalar
        def _(scalar):
            # dummy Copy at body start: Bacc places the 1.28us ACT_TABLE_LOAD
            # before it, hoisting the load into the input-DMA dead time
            scalar.copy(scr[0:1, 1:2], scr[0:1, 0:1])
            # half1: outputs 96:192 of each position live in PSUM partitions
            # 64:128.  Position 12's half1 is stale PSUM (chunk 21 has no
            # half1 outputs) -- host drops it.  Scalar-assigned output gens
            # are interleaved right after the unit that satisfies them, so
            # mid-stream groups fly while later units still extract.
            for u, (c0, c1) in enumerate(EX_UNITS):
                scalar.wait_ge(sem_pe, c1)
                scalar.copy(
                    st[B : 2 * B, c0:c1, 0:96],
                    pps[(c0 // 4) % 2].ap()[B : 2 * B, c0 % 4 : c0 % 4 + c1 - c0, 96:CT],
                ).then_inc(sem_act, 1)
                for gi, og in enumerate(OUT_GROUPS):
                    if og[3] == "scalar" and og[2] == u + 1:
                        out_dma(scalar, gi)

        @block.gpsimd
        def _(gpsimd):
            for gi, og in enumerate(OUT_GROUPS):
                if og[3] == "gpsimd":
                    out_dma(gpsimd, gi)
            for s in sem_out:
                gpsimd.wait_ge(s, 16)

        block_cm.__exit__(None, None, None)  # all-engine exit barrier
        # no explicit semaphore-clear block: the NRT postamble resets the
        # full user-semaphore range after the exit barrier on every exec

    return nc


def run_sharded(inputs, trace=False):
    global _prog_cache
    x = np.ascontiguousarray(np.asarray(inputs["input"], dtype=np.float32))
    xwt = _prep_inputs(x, inputs["alpha"], inputs["beta"])
    if _prog_cache is None:
        _prog_cache = _build_program()
    nc = _prog_cache
    in_maps = [{"xwt": xwt[cc]} for cc in range(NCORES)]
    res = run_bass_kernel_spmd(nc, in_maps, list(range(NCORES)), trace=trace)
    shards = []
    inv = np.array(P_INV)
    for cc in range(NCORES):
        yd = res.results[cc]["y"].reshape(2, B, NJ, 96)
        yd = yd[:, :, inv, :]  # position slots -> chunk order
        yc = np.transpose(yd, (1, 2, 0, 3)).reshape(B, NJ * CT)
        shards.append(yc[:, :TCORE].astype(np.float32))
    y = np.concatenate(shards, axis=1)
    return y, res


def kernel(input, alpha, beta):
    y, _ = run_sharded({"input": input, "alpha": alpha, "beta": beta})
    return y


# revision 4
# speedup vs baseline: 1.0015x; 1.0015x over previous
"""Time-varying 33-tap FIR low-pass filter on 8 Trainium2 NeuronCores.

y[b,t] = sum_u filt[t,u] * x[b, t+u-16],  filt = host-computed windowed-sinc
bank (n,33) derived from scalars alpha/beta (tiny; O(n*33) host FLOPs).

Sharding: time dim split across the 8 cores (4096 t-columns each, all 64
batch rows).  Per core the banded matmul y = x @ W (contraction over input
time s) is tiled into 22 TensorE matmuls.  Each matmul packs TWO 128-sample
x-chunks, offset by 96 samples, side by side in the stationary operand
(K=128, M=128 = 2 halves x 64 batch).  The 96-offset makes every output
column's 33-tap band land entirely inside one half, so each PSUM column is
valid in exactly one 64-row half and the chunk serves 192 output columns
with no accumulation pass:

  lhsT[k, 64*h + b] = x[b, S + 96*h + k]           (S = core_t0 - 16 + 192*j)
  rhs [k, n]        = filt[S+16+n, u] at k = (n % 96) + u   (zeros elsewhere)
  psum[64*h(n) + b, n] = y[b, S+16+n],   h(n) = n // 96

Measured structure (profile: exec window = first body instruction ->
last instruction, which INCLUDES the NRT postamble -- a fixed ~8.4us of
walrus exit barrier + 253 per-semaphore clears split across the 5 engines
(PE slowest, ~115ns each) + final barriers.  The controllable span is:
~1.5us body-start latency, the input DMA stream (~1.7MB at ~270GB/s --
a structural floor: the banded moving operand costs K*2B = 256B per
output column regardless of band content, and every on-device band-
generation scheme dies on either the partition-broadcast problem, per-op
engine fixed overheads, or DMA packet granularity), then a dependency
tail: last-input-byte -> completion sem (~0.5-0.9us) -> last matmuls ->
extraction -> output-DMA generation (~0.6us) -> DGE start delay (~0.66us)
-> transfer.

Schedule optimizations over the 22.2us/20.8us baseline:
- all matmul operands bf16; input partition-major, few LARGE grouped DMAs
  (as before)
- activation-table preload: a dummy scalar Copy at body start hoists the
  1.28us ACT_TABLE_LOAD off the first extraction's critical path
- PROCESSING ORDER [0..11, 21, 12..20]: the partial chunk (21: only 64
  valid outputs, 64-col matmul) rides mid-stream inside input group G1,
  so the final chain after the last input byte is exactly one full chunk
  (20): one matmul -> one pair of extractions -> one small gen+transfer
- finer input groups at the tail ((4),(4-11,21),(12-17),(18-19),(20)) so
  completion semaphores fire progressively and the PE never bunches
- output-DMA generations spread across THREE issue engines (gpsimd SWDGE
  for the first group, Sync and Scalar HWDGE alternating for the rest) --
  the baseline serialized all gens on Sync (~600ns each + 660ns DGE
  delay), which alone stretched the tail by ~2us
- st staging keeps half1 in partitions 64:128 (128-line output DMAs);
  host unfolds positions->chunks and upcasts
Fixed costs that bound further gains: the NRT postamble (~7.3us: 253 sem
clears + barriers -- def.json already declares runtime_semaphore_count=3,
the reset length is runtime hygiene, not kernel-controllable), ~0.7us
walrus entry/register preamble, ~0.9us DMA completion-semaphore latency,
~1.25us gen+DGE-delay on the final output group.
"""

import sys
from contextlib import ExitStack

import numpy as np
import ml_dtypes

if "/opt/trn_rl_repo" not in sys.path:
    sys.path.insert(0, "/opt/trn_rl_repo")

from concourse import bass, mybir
from concourse.bass_utils import run_bass_kernel_spmd

N = 32768          # time length
B = 64             # batch
NCORES = 8
TCORE = N // NCORES            # 4096 output columns per core
CT = 192                       # output columns served per chunk
NJ = (TCORE + CT - 1) // CT    # 22 chunks per core (last one partial: 64 cols)
KP = 128                       # contraction rows per matmul
TAPS = 33
HALF = 16
W = 128 + CT                   # 320 columns per [stationary | moving] chunk

# processing order: partial chunk 21 (64 valid cols) runs mid-stream so the
# final dependency chain covers exactly one chunk (20)
P_ORDER = list(range(0, 12)) + [21] + list(range(12, 21))
P_INV = [0] * NJ
for _p, _j in enumerate(P_ORDER):
    P_INV[_j] = _p

# input groups: lists of xwt column ranges (in chunk units), one semaphore
# each.  Chunk 21 (stationary + 64-col band, 192 cols -- its last 128 band
# cols are all-zero padding, never read) rides on its OWN semaphore right
# after G0, so neither chunks 4-11 nor the mid-stream chain wait on it and
# position 12's matmul is ready long before the PE reaches it.  Mid/tail
# groups are small so completion sems fire progressively and the PE never
# bunches after the stream drains.
IN_GROUPS = (
    ((0, 4, W),),          # G0: chunks 0-3
    ((21, 22, 192),),      # G1: chunk 21 (48KB, early)
    ((4, 12, W),),         # G2: chunks 4-11
    ((12, 15, W),),        # G3: chunks 12-14
    ((15, 18, W),),        # G4: chunks 15-17
    ((18, 20, W),),        # G5: chunks 18-19
    ((20, 21, W),),        # G6: chunk 20 (stream tail)
)
# input group index gating each POSITION's matmul
POS_GROUP = [0] * 4 + [2] * 8 + [1] + [3] * 3 + [4] * 3 + [5] * 2 + [6]
# extraction units over positions: quads while the stream runs, pairs and
# singles at the tail so the last unit (position 21 = chunk 20) is minimal
EX_UNITS = ((0, 4), (4, 8), (8, 12), (12, 14), (14, 16), (16, 18), (18, 20),
            (20, 21), (21, 22))
# output groups over positions: (pos0, pos1, unit threshold, issue engine).
# Mid groups gen on Sync: its HWDGE queue is FIFO behind the input stream,
# so their transfers are inherently just-in-time and never steal bus from
# the input.  The final tiny group (chunk 20) gens on Scalar immediately
# after its own last extraction -- a parallel queue, so it never waits on
# Sync's gen serialization.  Scalar issues NO mid-stream gens (a gen there
# delays every later extraction on the tail-critical engine).
OUT_GROUPS = (
    (0, 4, 1, "gpsimd"),
    (4, 12, 3, "sync"),
    (12, 18, 6, "sync"),
    (18, 21, 8, "sync"),
    (21, 22, 9, "scalar"),   # final: chunk 20 alone, 24KB
)

_prog_cache = None


def _filters_np(alpha, beta):
    """Numpy port of reference._filters (returns the flipped bank)."""
    t = np.arange(N, dtype=np.float64)
    cutoff = (np.pi / 4.0 + float(alpha) * np.sin(float(beta) * t / 8000.0)) / (
        2.0 * np.pi
    )
    k = np.arange(TAPS, dtype=np.float64)
    window = 0.5 - 0.5 * np.cos(2.0 * np.pi * k / (TAPS - 1.0))
    tvec = np.arange(-HALF, HALF + 1, dtype=np.float64)
    arg = 2.0 * np.pi * cutoff[:, None] * tvec[None, :]
    safe = np.where(arg == 0.0, 1.0, arg)
    sinc = np.where(arg == 0.0, 1.0, np.sin(safe) / safe)
    f = 2.0 * cutoff[:, None] * window[None, :] * sinc
    f = f / f.sum(axis=-1, keepdims=True)
    return np.ascontiguousarray(f[:, ::-1]).astype(np.float32)


def _prep_inputs(x, alpha, beta):
    """Build per-core [KP, NJ*W] bf16 [stationary | banded-filter] tiles."""
    filt = _filters_np(alpha, beta)  # (N, 33)

    pad = 16 + N + 512
    xp = np.zeros((B, pad), dtype=np.float32)
    xp[:, 16 : 16 + N] = x
    xp = xp.astype(ml_dtypes.bfloat16)
    fp = np.zeros((N + 512, TAPS), dtype=ml_dtypes.bfloat16)
    fp[:N] = filt.astype(ml_dtypes.bfloat16)

    c = np.arange(NCORES)[:, None, None, None]
    j = np.arange(NJ)[None, :, None, None]
    h = np.arange(2)[None, None, :, None]
    k = np.arange(KP)[None, None, None, :]
    # global s = TCORE*c - 16 + CT*j + 96*h + k ; +16 shifts into xp coords
    sidx = TCORE * c + CT * j + 96 * h + k
    xw = xp[:, sidx]  # (B, NCORES, NJ, 2, KP)
    xw = np.ascontiguousarray(
        np.transpose(xw, (1, 2, 4, 3, 0)).reshape(NCORES, NJ, KP, 128)
    )

    u = np.arange(TAPS)[:, None]  # (33, 1)
    nn = np.arange(CT)[None, :]  # (1, 192)
    rows = (nn % 96) + u  # (33, 192) target partition rows
    cols = np.broadcast_to(nn, (TAPS, CT))
    tg = (
        TCORE * np.arange(NCORES)[:, None, None]
        + CT * np.arange(NJ)[None, :, None]
        + np.arange(CT)[None, None, :]
    )  # (NCORES, NJ, 192) global output t per column
    vals = np.transpose(fp[tg], (0, 1, 3, 2))  # (NCORES, NJ, 33, 192)
    wt = np.zeros((NCORES, NJ, KP, CT), dtype=ml_dtypes.bfloat16)
    wt[:, :, rows, cols] = vals

    # one combined [stationary | moving] tile per chunk, then partition-major
    # ([KP, NJ, W]) so grouped input DMAs move long contiguous lines
    xwt = np.concatenate([xw, wt], axis=3)  # (NCORES, NJ, KP, W)
    xwt = np.transpose(xwt, (0, 2, 1, 3)).reshape(NCORES, KP, NJ * W)
    return np.ascontiguousarray(xwt)


def _build_program():
    """Raw Bass (no Tile): walrus permits a single sync-wait slot per Matmult
    and per DMA descriptor, so waits are emitted as standalone EventSemaphore
    instructions on each engine's queue instead."""
    # Skip the const-AP publish barrier at the tail of Bass.__init__: this
    # kernel never reads const_aps (scalar Copy keeps a float bias), the NRT
    # pseudo-barrier earlier in init already rendezvoused the engines, and
    # per-sem waits order everything else.  Saves ~0.3us of preamble.
    orig_aeb = bass.Bass.all_engine_barrier
    bass.Bass.all_engine_barrier = lambda self, *, sem_only=False: None
    try:
        nc = bass.Bass(trn_type="TRN2", debug=False)
    finally:
        bass.Bass.all_engine_barrier = orig_aeb
    f32 = mybir.dt.float32
    bf16 = mybir.dt.bfloat16
    xwt_d = nc.dram_tensor("xwt", [KP, NJ * W], bf16, kind="ExternalInput").ap()
    y_d = nc.dram_tensor("y", [2 * B, NJ * 96], bf16, kind="ExternalOutput").ap()

    def unit_of(p):
        for u, (c0, c1) in enumerate(EX_UNITS):
            if c0 <= p < c1:
                return u

    with ExitStack() as ctx:
        xts = ctx.enter_context(nc.sbuf_tensor("xts", [KP, NJ * W], bf16))
        # staging keeps half1 in partitions 64:128 (no partition fold), so
        # output DMAs move 128 lines instead of 64 -- better engine pipelining
        st = ctx.enter_context(nc.sbuf_tensor("st", [2 * B, NJ, 96], bf16))
        # tiny scratch for the activation-table preload copy
        scr = ctx.enter_context(nc.sbuf_tensor("scr", [1, 2], bf16))
        # 2 quad-tensors of 4 PSUM banks each (slot = 512 fp32 = one bank);
        # extraction reads all slots of a quad in one 3D-AP op
        pps = [
            ctx.enter_context(nc.psum_tensor(f"pp{i}", [128, 4, 512], f32))
            for i in range(2)
        ]
        # one semaphore per input DMA group (no reuse -> no guards needed)
        sem_in = [
            ctx.enter_context(nc.semaphore(f"s_in{i}"))
            for i in range(len(IN_GROUPS))
        ]
        sem_pe = ctx.enter_context(nc.semaphore("s_pe"))
        sem_dve = ctx.enter_context(nc.semaphore("s_dve"))
        sem_act = ctx.enter_context(nc.semaphore("s_act"))
        sem_out = [
            ctx.enter_context(nc.semaphore(f"s_out{i}"))
            for i in range(len(OUT_GROUPS))
        ]
        block_cm = nc.Block()
        block = block_cm.__enter__()

        # expected completion value per input group (16 per dma_start)
        in_target = [16 * len(g) for g in IN_GROUPS]

        def out_dma(eng, gi):
            p0, p1, uth, _ = OUT_GROUPS[gi]
            eng.wait_ge(sem_dve, uth)
            eng.wait_ge(sem_act, uth)
            eng.dma_start(
                out=y_d[:, 96 * p0 : 96 * p1], in_=st[:, p0:p1, :]
            ).then_inc(sem_out[gi], 16)

        @block.sync
        def _(sync):
            # all input groups from one engine: generation order == transfer
            # order, so group 0 always reaches the DMA engines first
            for g, ranges in enumerate(IN_GROUPS):
                for (j0, j1, w1) in ranges:
                    sync.dma_start(
                        out=xts[:, W * j0 : W * (j1 - 1) + w1],
                        in_=xwt_d[:, W * j0 : W * (j1 - 1) + w1],
                    ).then_inc(sem_in[g], 16)
            for gi, og in enumerate(OUT_GROUPS):
                if og[3] == "sync":
                    out_dma(sync, gi)

        @block.tensor
        def _(tensor):
            reuse_th = 0
            seen_group = -1
            for p, j in enumerate(P_ORDER):
                g = POS_GROUP[p]
                if g != seen_group:
                    tensor.wait_ge(sem_in[g], in_target[g])
                    seen_group = g
                if p >= 8:
                    # PSUM slot of position p (bank p%8) is free once the unit
                    # holding position p-8 has BOTH half-copies retired
                    th = unit_of(p - 8) + 1
                    if th > reuse_th:
                        tensor.wait_ge(sem_dve, th)
                        tensor.wait_ge(sem_act, th)
                        reuse_th = th
                ncols = 64 if j == NJ - 1 else CT
                tensor.matmul(
                    pps[(p // 4) % 2].ap()[:, p % 4, 0:ncols],
                    xts[:, W * j : W * j + 128],
                    xts[:, W * j + 128 : W * j + 128 + ncols],
                    start=True,
                    stop=True,
                ).then_inc(sem_pe, 1)

        @block.vector
        def _(vector):
            # half0: outputs 0:96 of each position live in PSUM partitions
            # 0:64; one 3D-AP op extracts a whole unit.  For position 12
            # (chunk 21, 64-col matmul) columns 64:96 read stale PSUM -- the
            # host drops those outputs (beyond TCORE).
            for u, (c0, c1) in enumerate(EX_UNITS):
                vector.wait_ge(sem_pe, c1)
                vector.tensor_copy(
                    st[0:B, c0:c1, 0:96],
                    pps[(c0 // 4) % 2].ap()[0:B, c0 % 4 : c0 % 4 + c1 - c0, 0:96],
                ).then_inc(sem_dve, 1)

        @block.scalar
        def _(scalar):
            # dummy Copy at body start: Bacc places the 1.28us ACT_TABLE_LOAD
            # before it, hoisting the load into the input-DMA dead time
            scalar.copy(scr[0:1, 1:2], scr[0:1, 0:1])
            # half1: outputs 96:192 of each position live in PSUM partitions
            # 64:128.  Position 12's half1 is stale PSUM (chunk 21 has no
            # half1 outputs) -- host drops it.  Scalar-assigned output gens
            # are interleaved right after the unit that satisfies them, so
            # mid-stream groups fly while later units still extract.
            for u, (c0, c1) in enumerate(EX_UNITS):
                scalar.wait_ge(sem_pe, c1)
                scalar.copy(
                    st[B : 2 * B, c0:c1, 0:96],
                    pps[(c0 // 4) % 2].ap()[B : 2 * B, c0 % 4 : c0 % 4 + c1 - c0, 96:CT],
                ).then_inc(sem_act, 1)
                for gi, og in enumerate(OUT_GROUPS):
                    if og[3] == "scalar" and og[2] == u + 1:
                        out_dma(scalar, gi)

        @block.gpsimd
        def _(gpsimd):
            for gi, og in enumerate(OUT_GROUPS):
                if og[3] == "gpsimd":
                    out_dma(gpsimd, gi)
            for s in sem_out:
                gpsimd.wait_ge(s, 16)

        block_cm.__exit__(None, None, None)  # all-engine exit barrier
        # no explicit semaphore-clear block: the NRT postamble resets the
        # full user-semaphore range after the exit barrier on every exec

    return nc


def run_sharded(inputs, trace=False):
    global _prog_cache
    x = np.ascontiguousarray(np.asarray(inputs["input"], dtype=np.float32))
    xwt = _prep_inputs(x, inputs["alpha"], inputs["beta"])
    if _prog_cache is None:
        _prog_cache = _build_program()
    nc = _prog_cache
    in_maps = [{"xwt": xwt[cc]} for cc in range(NCORES)]
    res = run_bass_kernel_spmd(nc, in_maps, list(range(NCORES)), trace=trace)
    shards = []
    inv = np.array(P_INV)
    for cc in range(NCORES):
        yd = res.results[cc]["y"].reshape(2, B, NJ, 96)
        yd = yd[:, :, inv, :]  # position slots -> chunk order
        yc = np.transpose(yd, (1, 2, 0, 3)).reshape(B, NJ * CT)
        shards.append(yc[:, :TCORE].astype(np.float32))
    y = np.concatenate(shards, axis=1)
    return y, res


def kernel(input, alpha, beta):
    y, _ = run_sharded({"input": input, "alpha": alpha, "beta": beta})
    return y
